# revision 11
# baseline (speedup 1.0000x reference)
"""Trainium2 Bass kernel for BertSelfAttention (B=1, S=4096, HID=768, 12 heads).

Sharding: 8 cores = 4 head-groups x 2 query-halves. Each core computes 3 heads
for 2048 query rows against all 4096 keys, fused (scores never hit HBM).

Host-side sharding prep packs each core's inputs in their on-chip layout
(bf16, transposed hidden states, chunk-major weights, duplicated biases), so
the device spends no time on layout transforms.

Per-core dataflow (bf16 matmuls, fp32 PSUM accumulation):
  - Q^T per head duplicated on both partition halves; K^T per head stored with
    even/odd key-chunks on partition halves -> score matmuls (contraction=64)
    run as row-tiled concurrent pairs at full PE rate.
  - scores computed transposed: S^T[k, q] tiles land in PSUM; one ScalarE Exp
    instruction per [128, 1024] tile writes bf16 P^T straight to SBUF
    (scale=1/8 folded into the activation).
  - additive attention mask handled exactly by scaling V rows (and the
    appended ones-column) with exp(mask[k]) computed on device.
  - V is augmented with a ones column per head, so the context matmul
    accumulates both sum(p*v) and sum(p) (the softmax denominator) in one
    PSUM group.
  - ctx^T [65, 512] tiles are PE-transposed back to [q, d] layout, divided by
    the denominator on VectorE, and DMA'd out.
  - projection/V work is hand-interleaved into the ScalarE-bound attention
    sweep so the PE fills activation bubbles instead of serializing up front.
"""

import sys

sys.path.insert(0, "/opt/trn_rl_repo")

import ml_dtypes
import numpy as np

import concourse.bacc as bacc
import concourse.mybir as mybir
import concourse.tile as tile
from concourse import bass_utils

B, S, HID = 1, 4096, 768
NH, HD = 12, 64
N_CORES = 8
HG = 4  # head-groups (tensor parallel)
QS = 2  # query splits (data parallel on sequence)
HPC = NH // HG  # 3 heads per core
SQ = S // QS  # 2048 query rows per core
CC = HPC * HD  # 192 projection columns per core
VC = HPC * (HD + 1)  # 195 augmented V columns (ones col per head)
NHC = HID // 128  # 6 contraction chunks
NT = S // 128  # 32 key tiles
NB = 4  # s-blocks for input pipelining (1024 rows each)
SB = S // NB  # 1024

f32 = mybir.dt.float32
bf16 = mybir.dt.bfloat16
bf16np = ml_dtypes.bfloat16

_CACHE = {}


def _build():
    EXP = mybir.ActivationFunctionType.Exp
    nc = bacc.Bacc("TRN2", target_bir_lowering=False)

    hsT_d = nc.dram_tensor("hsT", [HID, S], bf16, kind="ExternalInput")
    hsqT_d = nc.dram_tensor("hsqT", [HID, SQ], bf16, kind="ExternalInput")
    wqb_d = nc.dram_tensor("wqb", [128, NHC * CC], bf16, kind="ExternalInput")
    wkb_d = nc.dram_tensor("wkb", [128, NHC * CC], bf16, kind="ExternalInput")
    wvb_d = nc.dram_tensor("wvb", [128, NHC * VC], bf16, kind="ExternalInput")
    bqt_d = nc.dram_tensor("bqt", [128, HPC], f32, kind="ExternalInput")
    bkt_d = nc.dram_tensor("bkt", [128, HPC], f32, kind="ExternalInput")
    bvb_d = nc.dram_tensor("bvb", [1, VC], bf16, kind="ExternalInput")
    maskt_d = nc.dram_tensor("maskt", [128, NT], f32, kind="ExternalInput")
    ident_d = nc.dram_tensor("ident", [128, 128], f32, kind="ExternalInput")
    out_d = nc.dram_tensor("out", [SQ, CC], f32, kind="ExternalOutput")

    with tile.TileContext(nc) as tc:
        with (
            tc.tile_pool(name="persist", bufs=1) as P,
            tc.tile_pool(name="work", bufs=3) as WK,
            tc.tile_pool(name="outp", bufs=2) as OP,
            tc.tile_pool(name="ppsum", bufs=2, space="PSUM") as PP,
            tc.tile_pool(name="bpsum", bufs=2, space="PSUM") as BP,
            tc.tile_pool(name="cpsum", bufs=2, space="PSUM") as CP,
        ):
            # ---- persistent SBUF tensors ----
            # chunk-major transposed activations: chunk c at cols [c*S, (c+1)*S)
            hsT = P.tile([128, NHC * S], bf16, tag="hsT")
            hsTq = P.tile([128, NHC * SQ], bf16, tag="hsTq")
            wqb = P.tile([128, NHC * CC], bf16, tag="wqb")
            wkb = P.tile([128, NHC * CC], bf16, tag="wkb")
            wvb = P.tile([128, NHC * VC], bf16, tag="wvb")
            bvb = P.tile([1, VC], bf16, tag="bvb")
            bqt = P.tile([128, HPC], f32, tag="bqt")
            bkt = P.tile([128, HPC], f32, tag="bkt")
            maskt = P.tile([128, NT], f32, tag="maskt")
            wmask = P.tile([128, NT], f32, tag="wmask")
            identf = P.tile([128, 128], f32, tag="identf")
            onesb = P.tile([1, 128], bf16, tag="onesb")
            qt = [
                P.tile([128, SQ], bf16, tag=f"qt{h}", name=f"qt{h}")
                for h in range(HPC)
            ]
            kt = [
                P.tile([128, S // 2], bf16, tag=f"kt{h}", name=f"kt{h}")
                for h in range(HPC)
            ]
            vv = P.tile([128, NT * VC], bf16, tag="vv")


            # ---- emission helpers ----
            # batched multi-chunk loads: one DMA covers all 6 contraction
            # chunks for a column range (3-dim access pattern)
            hsT_3d = hsT.rearrange("p (c s) -> p c s", s=S)
            hsT_d3 = hsT_d.rearrange("(c p) s -> p c s", p=128)
            hsTq_3d = hsTq.rearrange("p (c s) -> p c s", s=SQ)
            hsqT_d3 = hsqT_d.rearrange("(c p) s -> p c s", p=128)

            def load_hsT_cols(s0, s1):
                nc.sync.dma_start(hsT_3d[:, :, s0:s1], hsT_d3[:, :, s0:s1])

            def load_hsqT_cols(s0, s1):
                nc.sync.dma_start(hsTq_3d[:, :, s0:s1], hsqT_d3[:, :, s0:s1])

            qt_done = set()

            def qt_unit(h, j):
                # produces qt[h][:, j*512:(j+1)*512], duplicated on both
                # partition halves
                if (h, j) in qt_done:
                    return
                qt_done.add((h, j))
                pq = PP.tile([128, 512], f32, tag="proj", name="pq")
                for c in range(NHC):
                    lw = wqb[:, c * CC + h * 64 : c * CC + (h + 1) * 64]
                    rq = hsTq[:, c * SQ + j * 512 : c * SQ + (j + 1) * 512]
                    nc.tensor.matmul(
                        pq[0:64, :], lw, rq, start=(c == 0), stop=(c == NHC - 1)
                    )
                    nc.tensor.matmul(
                        pq[64:128, :], lw, rq, start=(c == 0), stop=(c == NHC - 1)
                    )
                nc.vector.tensor_scalar_add(
                    qt[h][:, j * 512 : (j + 1) * 512], pq[:], bqt[:, h : h + 1]
                )

            kt_done = set()

            def kt_unit(h, j):
                # produces kt[h][:, j*512:(j+1)*512]: key chunks 8j..8j+7,
                # even chunks on partitions 0:64, odd on 64:128
                if (h, j) in kt_done:
                    return
                kt_done.add((h, j))
                pk = PP.tile([128, 512], f32, tag="proj", name="pk")
                for c in range(NHC):
                    lw = wkb[:, c * CC + h * 64 : c * CC + (h + 1) * 64]
                    base = hsT[:, c * S + j * SB : c * S + (j + 1) * SB]
                    eo = base.rearrange("p (t two x) -> p t two x", two=2, x=128)
                    nc.tensor.matmul(
                        pk[0:64, :],
                        lw,
                        eo[:, :, 0, :],
                        start=(c == 0),
                        stop=(c == NHC - 1),
                    )
                    nc.tensor.matmul(
                        pk[64:128, :],
                        lw,
                        eo[:, :, 1, :],
                        start=(c == 0),
                        stop=(c == NHC - 1),
                    )
                nc.vector.tensor_scalar_add(
                    kt[h][:, j * 512 : (j + 1) * 512], pk[:], bkt[:, h : h + 1]
                )

            def v_unit(t):
                pv = PP.tile([128, VC], f32, tag="proj", name="pv")
                for c in range(NHC):
                    nc.tensor.matmul(
                        pv[:],
                        hsT[:, c * S + t * 128 : c * S + (t + 1) * 128],
                        wvb[:, c * VC : (c + 1) * VC],
                        start=(c == 0),
                        stop=False,
                    )
                nc.tensor.matmul(pv[:], onesb[:], bvb[:], start=False, stop=True)
                nc.vector.tensor_scalar_mul(
                    vv[:, t * VC : (t + 1) * VC], pv[:], wmask[:, t : t + 1]
                )

            # ---- ramp: pipelined input loads + first-needed projections ----
            load_hsqT_cols(0, 512)  # enough for qt(*, 0)
            load_hsT_cols(0, 1024)
            nc.sync.dma_start(wqb[:], wqb_d[:])
            nc.sync.dma_start(bqt[:], bqt_d[:])
            nc.sync.dma_start(wkb[:], wkb_d[:])
            nc.sync.dma_start(bkt[:], bkt_d[:])
            qt_unit(0, 0)
            kt_unit(0, 0)
            load_hsT_cols(1024, 2048)
            nc.sync.dma_start(wvb[:], wvb_d[:])
            nc.sync.dma_start(bvb[:], bvb_d[:])
            nc.sync.dma_start(maskt[:], maskt_d[:])
            nc.vector.memset(onesb[:], 1.0)
            nc.scalar.activation(wmask[:], maskt[:], EXP)
            nc.sync.dma_start(identf[:], ident_d[:])
            load_hsT_cols(2048, 4096)
            load_hsqT_cols(512, SQ)

            # deferred out-stage, pipelined into the next block's g-loop
            out_stage_q = []

            def emit_out_stage():
                # step 0: copy PSUM ctx -> SBUF; steps 1..4: transpose +
                # normalize + pack one 128-row tile; step 4 also stores
                if not out_stage_q:
                    return
                jq, h, cx, st = out_stage_q[0]
                if st["step"] == 0:
                    cs = OP.tile([65, 512], f32, tag="cs", name="cs")
                    nc.vector.tensor_copy(cs[:], cx[:])
                    st["cs"] = cs
                    st["ot"] = OP.tile([128, 4 * 64], f32, tag="ot", name="ot")
                elif st["step"] <= 4:
                    t4 = st["step"] - 1
                    cs, ot = st["cs"], st["ot"]
                    tp2 = PP.tile([128, 65], f32, tag="proj", name="tp2")
                    nc.tensor.transpose(
                        tp2[:], cs[:, t4 * 128 : (t4 + 1) * 128], identf[0:65, 0:65]
                    )
                    rc = OP.tile([128, 1], f32, tag="rc", name="rc")
                    nc.vector.reciprocal(rc[:], tp2[:, 64:65])
                    nc.vector.tensor_scalar_mul(
                        ot[:, t4 * 64 : (t4 + 1) * 64], tp2[:, 0:64], rc[:]
                    )
                    if t4 == 3:
                        dst = out_d[
                            jq * 512 : (jq + 1) * 512, h * 64 : (h + 1) * 64
                        ].rearrange("(t p) d -> p t d", p=128)
                        nc.sync.dma_start(
                            dst, ot.rearrange("p (t d) -> p t d", d=64)
                        )
                        out_stage_q.pop(0)
                        return
                st["step"] += 1

            def flush_out_stages():
                while out_stage_q:
                    emit_out_stage()

            # ---- attention sweep (head-outer for projection spreading) ----
            blocks = [(jq, h) for h in range(HPC) for jq in range(SQ // 512)]
            kt_queue = [(1, j) for j in range(4)] + [(2, j) for j in range(4)]

            for bi, (jq, h) in enumerate(blocks):
                qt_unit(h, jq)
                cx = CP.tile([65, 512], f32, tag="ctx", name="cx")
                for g in range(16):
                    emit_out_stage()
                    # interleave remaining projection work into the
                    # ScalarE-bound steady state
                    if bi == 0:
                        v_unit(2 * g)
                        v_unit(2 * g + 1)
                        if g in (0, 4, 8):
                            kt_unit(0, g // 4 + 1)
                    elif g in (5, 11) and kt_queue:
                        kt_unit(*kt_queue.pop(0))
                    if g == 8 and bi + 1 < len(blocks):
                        njq, nh = blocks[bi + 1]
                        qt_unit(nh, njq)
                    if g == 12 and bi + 2 < len(blocks):
                        njq, nh = blocks[bi + 2]
                        qt_unit(nh, njq)

                    sc = BP.tile([128, 1024], f32, tag="big", name="sc")
                    nc.tensor.matmul(
                        sc[:, 0:512],
                        kt[h][0:64, g * 128 : (g + 1) * 128],
                        qt[h][0:64, jq * 512 : (jq + 1) * 512],
                        start=True,
                        stop=True,
                    )
                    nc.tensor.matmul(
                        sc[:, 512:1024],
                        kt[h][64:128, g * 128 : (g + 1) * 128],
                        qt[h][64:128, jq * 512 : (jq + 1) * 512],
                        start=True,
                        stop=True,
                    )
                    pt = WK.tile([128, 1024], bf16, tag="pts", name="pt")
                    nc.scalar.activation(pt[:], sc[:], EXP, scale=0.125)
                    nc.tensor.matmul(
                        cx[:],
                        vv[:, (2 * g) * VC + h * 65 : (2 * g) * VC + h * 65 + 65],
                        pt[:, 0:512],
                        start=(g == 0),
                        stop=False,
                    )
                    nc.tensor.matmul(
                        cx[:],
                        vv[
                            :,
                            (2 * g + 1) * VC
                            + h * 65 : (2 * g + 1) * VC
                            + h * 65
                            + 65,
                        ],
                        pt[:, 512:1024],
                        start=False,
                        stop=(g == 15),
                    )
                out_stage_q.append((jq, h, cx, {"step": 0}))
            flush_out_stages()

    nc.compile()
    return nc


def _get_nc():
    if "nc" not in _CACHE:
        _CACHE["nc"] = _build()
    return _CACHE["nc"]


def _in_maps(hs, mask, Wq, bq, Wk, bk, Wv, bv):
    ident = np.eye(128, dtype=np.float32)
    maskt = np.ascontiguousarray(mask.reshape(NT, 128).T)  # [128, 32]
    hsT = np.ascontiguousarray(hs.astype(bf16np).T)  # [768, 4096] bf16
    hsqT = [
        np.ascontiguousarray(hs[sh * SQ : (sh + 1) * SQ, :].astype(bf16np).T)
        for sh in range(QS)
    ]

    def chunk_major(W, cols):  # [768, ncols] f32 -> [128, 6*ncols] bf16
        out = np.empty((128, NHC * cols), bf16np)
        for c in range(NHC):
            out[:, c * cols : (c + 1) * cols] = W[c * 128 : (c + 1) * 128, :].astype(
                bf16np
            )
        return out

    maps = []
    for core in range(N_CORES):
        hg, sh = core // QS, core % QS
        csl = slice(hg * CC, (hg + 1) * CC)
        wv_aug = np.zeros((HID, VC), np.float32)
        bv_aug = np.zeros((1, VC), np.float32)
        for h in range(HPC):
            wv_aug[:, h * 65 : h * 65 + 64] = Wv[
                :, hg * CC + h * 64 : hg * CC + (h + 1) * 64
            ]
            bv_aug[0, h * 65 : h * 65 + 64] = bv[
                hg * CC + h * 64 : hg * CC + (h + 1) * 64
            ]
            bv_aug[0, h * 65 + 64] = 1.0
        bqt = np.empty((128, HPC), np.float32)
        bkt = np.empty((128, HPC), np.float32)
        for h in range(HPC):
            bqt[0:64, h] = bq[hg * CC + h * 64 : hg * CC + (h + 1) * 64]
            bqt[64:128, h] = bqt[0:64, h]
            bkt[0:64, h] = bk[hg * CC + h * 64 : hg * CC + (h + 1) * 64]
            bkt[64:128, h] = bkt[0:64, h]
        maps.append(
            {
                "hsT": hsT,
                "hsqT": hsqT[sh],
                "wqb": chunk_major(Wq[:, csl], CC),
                "wkb": chunk_major(Wk[:, csl], CC),
                "wvb": chunk_major(wv_aug, VC),
                "bqt": bqt,
                "bkt": bkt,
                "bvb": bv_aug.astype(bf16np),
                "maskt": maskt,
                "ident": ident,
            }
        )
    return maps


def kernel(hidden_states, attention_mask, Wq, bq, Wk, bk, Wv, bv, **run_kwargs):
    hs = np.ascontiguousarray(np.asarray(hidden_states, np.float32).reshape(S, HID))
    mask = np.ascontiguousarray(np.asarray(attention_mask, np.float32).reshape(S))
    Wq = np.asarray(Wq, np.float32)
    Wk = np.asarray(Wk, np.float32)
    Wv = np.asarray(Wv, np.float32)
    bq = np.asarray(bq, np.float32)
    bk = np.asarray(bk, np.float32)
    bv = np.asarray(bv, np.float32)

    nc = _get_nc()
    maps = _in_maps(hs, mask, Wq, bq, Wk, bk, Wv, bv)
    res = bass_utils.run_bass_kernel_spmd(
        nc, maps, core_ids=list(range(N_CORES)), **run_kwargs
    )
    out = np.zeros((S, NH * HD), np.float32)
    for core in range(N_CORES):
        hg, sh = core // QS, core % QS
        out[sh * SQ : (sh + 1) * SQ, hg * CC : (hg + 1) * CC] = res.results[core][
            "out"
        ]
    if "trace" in run_kwargs:
        _CACHE["last_result"] = res
    return out.reshape(B, S, NH * HD)


# revision 13
# speedup vs baseline: 1.0291x; 1.0291x over previous
"""Trainium2 Bass kernel for BertSelfAttention (B=1, S=4096, HID=768, 12 heads).

Sharding: 8 cores = 4 head-groups x 2 query-halves. Each core computes 3 heads
for 2048 query rows against all 4096 keys, fused (scores never hit HBM).

Host-side sharding prep packs each core's inputs in their on-chip layout
(bf16, transposed hidden states, chunk-major weights, duplicated biases), so
the device spends no time on layout transforms.

Per-core dataflow (bf16 matmuls, fp32 PSUM accumulation):
  - Q^T per head duplicated on both partition halves; K^T per head stored with
    even/odd key-chunks on partition halves -> score matmuls (contraction=64)
    run as row-tiled concurrent pairs at full PE rate.
  - scores computed transposed: S^T[k, q] tiles land in PSUM; one ScalarE Exp
    instruction per [128, 1024] tile writes bf16 P^T straight to SBUF
    (scale=1/8 folded into the activation).
  - additive attention mask handled exactly by scaling V rows (and the
    appended ones-column) with exp(mask[k]) computed on device.
  - V is augmented with a ones column per head, so the context matmul
    accumulates both sum(p*v) and sum(p) (the softmax denominator) in one
    PSUM group.
  - ctx^T [65, 512] tiles are PE-transposed back to [q, d] layout, divided by
    the denominator on VectorE, and DMA'd out.
  - projection/V work is hand-interleaved into the ScalarE-bound attention
    sweep so the PE fills activation bubbles instead of serializing up front.
"""

import sys

sys.path.insert(0, "/opt/trn_rl_repo")

import ml_dtypes
import numpy as np

import concourse.bacc as bacc
import concourse.mybir as mybir
import concourse.tile as tile
from concourse import bass_utils

B, S, HID = 1, 4096, 768
NH, HD = 12, 64
N_CORES = 8
HG = 4  # head-groups (tensor parallel)
QS = 2  # query splits (data parallel on sequence)
HPC = NH // HG  # 3 heads per core
SQ = S // QS  # 2048 query rows per core
CC = HPC * HD  # 192 projection columns per core
VC = HPC * (HD + 1)  # 195 augmented V columns (ones col per head)
NHC = HID // 128  # 6 contraction chunks
NT = S // 128  # 32 key tiles
NB = 4  # s-blocks for input pipelining (1024 rows each)
SB = S // NB  # 1024

f32 = mybir.dt.float32
bf16 = mybir.dt.bfloat16
bf16np = ml_dtypes.bfloat16

_CACHE = {}


def _build():
    EXP = mybir.ActivationFunctionType.Exp
    nc = bacc.Bacc("TRN2", target_bir_lowering=False)

    hsT_d = nc.dram_tensor("hsT", [HID, S], bf16, kind="ExternalInput")
    hsqT_d = nc.dram_tensor("hsqT", [HID, SQ], bf16, kind="ExternalInput")
    wqb_d = nc.dram_tensor("wqb", [128, NHC * CC], bf16, kind="ExternalInput")
    wkb_d = nc.dram_tensor("wkb", [128, NHC * CC], bf16, kind="ExternalInput")
    wvb_d = nc.dram_tensor("wvb", [128, NHC * VC], bf16, kind="ExternalInput")
    bqt_d = nc.dram_tensor("bqt", [128, HPC], f32, kind="ExternalInput")
    bkt_d = nc.dram_tensor("bkt", [128, HPC], f32, kind="ExternalInput")
    bvb_d = nc.dram_tensor("bvb", [1, VC], bf16, kind="ExternalInput")
    maskt_d = nc.dram_tensor("maskt", [128, NT], f32, kind="ExternalInput")
    ident_d = nc.dram_tensor("ident", [128, 128], f32, kind="ExternalInput")
    out_d = nc.dram_tensor("out", [SQ, CC], f32, kind="ExternalOutput")

    with tile.TileContext(nc) as tc:
        with (
            tc.tile_pool(name="persist", bufs=1) as P,
            tc.tile_pool(name="work", bufs=4) as WK,
            tc.tile_pool(name="outp", bufs=2) as OP,
            tc.tile_pool(name="ppsum", bufs=2, space="PSUM") as PP,
            tc.tile_pool(name="bpsum", bufs=2, space="PSUM") as BP,
            tc.tile_pool(name="cpsum", bufs=2, space="PSUM") as CP,
        ):
            # ---- persistent SBUF tensors ----
            # chunk-major transposed activations: chunk c at cols [c*S, (c+1)*S)
            hsT = P.tile([128, NHC * S], bf16, tag="hsT")
            hsTq = P.tile([128, NHC * SQ], bf16, tag="hsTq")
            wqb = P.tile([128, NHC * CC], bf16, tag="wqb")
            wkb = P.tile([128, NHC * CC], bf16, tag="wkb")
            wvb = P.tile([128, NHC * VC], bf16, tag="wvb")
            bvb = P.tile([1, VC], bf16, tag="bvb")
            bqt = P.tile([128, HPC], f32, tag="bqt")
            bkt = P.tile([128, HPC], f32, tag="bkt")
            maskt = P.tile([128, NT], f32, tag="maskt")
            wmask = P.tile([128, NT], f32, tag="wmask")
            identf = P.tile([128, 128], f32, tag="identf")
            onesb = P.tile([1, 128], bf16, tag="onesb")
            qt = [
                P.tile([128, SQ], bf16, tag=f"qt{h}", name=f"qt{h}")
                for h in range(HPC)
            ]
            kt = [
                P.tile([128, S // 2], bf16, tag=f"kt{h}", name=f"kt{h}")
                for h in range(HPC)
            ]
            vv = P.tile([128, NT * VC], bf16, tag="vv")


            # ---- emission helpers ----
            # batched multi-chunk loads: one DMA covers all 6 contraction
            # chunks for a column range (3-dim access pattern)
            hsT_3d = hsT.rearrange("p (c s) -> p c s", s=S)
            hsT_d3 = hsT_d.rearrange("(c p) s -> p c s", p=128)
            hsTq_3d = hsTq.rearrange("p (c s) -> p c s", s=SQ)
            hsqT_d3 = hsqT_d.rearrange("(c p) s -> p c s", p=128)

            def load_hsT_cols(s0, s1):
                nc.sync.dma_start(hsT_3d[:, :, s0:s1], hsT_d3[:, :, s0:s1])

            def load_hsqT_cols(s0, s1):
                nc.sync.dma_start(hsTq_3d[:, :, s0:s1], hsqT_d3[:, :, s0:s1])

            qt_done = set()

            def qt_unit(h, j):
                # produces qt[h][:, j*512:(j+1)*512], duplicated on both
                # partition halves
                if (h, j) in qt_done:
                    return
                qt_done.add((h, j))
                pq = PP.tile([128, 512], f32, tag="proj", name="pq")
                for c in range(NHC):
                    lw = wqb[:, c * CC + h * 64 : c * CC + (h + 1) * 64]
                    rq = hsTq[:, c * SQ + j * 512 : c * SQ + (j + 1) * 512]
                    nc.tensor.matmul(
                        pq[0:64, :], lw, rq, start=(c == 0), stop=(c == NHC - 1)
                    )
                    nc.tensor.matmul(
                        pq[64:128, :], lw, rq, start=(c == 0), stop=(c == NHC - 1)
                    )
                nc.vector.tensor_scalar_add(
                    qt[h][:, j * 512 : (j + 1) * 512], pq[:], bqt[:, h : h + 1]
                )

            kt_done = set()

            def kt_unit(h, j):
                # produces kt[h][:, j*512:(j+1)*512]: key chunks 8j..8j+7,
                # even chunks on partitions 0:64, odd on 64:128
                if (h, j) in kt_done:
                    return
                kt_done.add((h, j))
                pk = PP.tile([128, 512], f32, tag="proj", name="pk")
                for c in range(NHC):
                    lw = wkb[:, c * CC + h * 64 : c * CC + (h + 1) * 64]
                    base = hsT[:, c * S + j * SB : c * S + (j + 1) * SB]
                    eo = base.rearrange("p (t two x) -> p t two x", two=2, x=128)
                    nc.tensor.matmul(
                        pk[0:64, :],
                        lw,
                        eo[:, :, 0, :],
                        start=(c == 0),
                        stop=(c == NHC - 1),
                    )
                    nc.tensor.matmul(
                        pk[64:128, :],
                        lw,
                        eo[:, :, 1, :],
                        start=(c == 0),
                        stop=(c == NHC - 1),
                    )
                nc.vector.tensor_scalar_add(
                    kt[h][:, j * 512 : (j + 1) * 512], pk[:], bkt[:, h : h + 1]
                )

            def v_unit(t):
                pv = PP.tile([128, VC], f32, tag="proj", name="pv")
                for c in range(NHC):
                    nc.tensor.matmul(
                        pv[:],
                        hsT[:, c * S + t * 128 : c * S + (t + 1) * 128],
                        wvb[:, c * VC : (c + 1) * VC],
                        start=(c == 0),
                        stop=False,
                    )
                nc.tensor.matmul(pv[:], onesb[:], bvb[:], start=False, stop=True)
                nc.vector.tensor_scalar_mul(
                    vv[:, t * VC : (t + 1) * VC], pv[:], wmask[:, t : t + 1]
                )

            # ---- ramp: pipelined input loads + first-needed projections ----
            load_hsqT_cols(0, 512)  # enough for qt(*, 0)
            load_hsT_cols(0, 1024)
            nc.sync.dma_start(wqb[:], wqb_d[:])
            nc.sync.dma_start(bqt[:], bqt_d[:])
            nc.sync.dma_start(wkb[:], wkb_d[:])
            nc.sync.dma_start(bkt[:], bkt_d[:])
            qt_unit(0, 0)
            kt_unit(0, 0)
            load_hsT_cols(1024, 2048)
            nc.sync.dma_start(wvb[:], wvb_d[:])
            nc.sync.dma_start(bvb[:], bvb_d[:])
            nc.sync.dma_start(maskt[:], maskt_d[:])
            nc.vector.memset(onesb[:], 1.0)
            nc.scalar.activation(wmask[:], maskt[:], EXP)
            nc.sync.dma_start(identf[:], ident_d[:])
            load_hsT_cols(2048, 4096)
            load_hsqT_cols(512, SQ)

            # deferred out-stage, pipelined into the next block's g-loop
            out_stage_q = []

            def emit_out_stage():
                # step 0: copy PSUM ctx -> SBUF; steps 1..4: transpose +
                # normalize + pack one 128-row tile; step 4 also stores
                if not out_stage_q:
                    return
                jq, h, cx, st = out_stage_q[0]
                if st["step"] == 0:
                    cs = OP.tile([65, 512], f32, tag="cs", name="cs")
                    nc.vector.tensor_copy(cs[:], cx[:])
                    st["cs"] = cs
                    st["ot"] = OP.tile([128, 4 * 64], f32, tag="ot", name="ot")
                elif st["step"] <= 4:
                    t4 = st["step"] - 1
                    cs, ot = st["cs"], st["ot"]
                    tp2 = PP.tile([128, 65], f32, tag="proj", name="tp2")
                    nc.tensor.transpose(
                        tp2[:], cs[:, t4 * 128 : (t4 + 1) * 128], identf[0:65, 0:65]
                    )
                    rc = OP.tile([128, 1], f32, tag="rc", name="rc")
                    nc.vector.reciprocal(rc[:], tp2[:, 64:65])
                    nc.vector.tensor_scalar_mul(
                        ot[:, t4 * 64 : (t4 + 1) * 64], tp2[:, 0:64], rc[:]
                    )
                    if t4 == 3:
                        dst = out_d[
                            jq * 512 : (jq + 1) * 512, h * 64 : (h + 1) * 64
                        ].rearrange("(t p) d -> p t d", p=128)
                        nc.sync.dma_start(
                            dst, ot.rearrange("p (t d) -> p t d", d=64)
                        )
                        out_stage_q.pop(0)
                        return
                st["step"] += 1

            def flush_out_stages():
                while out_stage_q:
                    emit_out_stage()

            # ---- attention sweep (head-outer for projection spreading) ----
            blocks = [(jq, h) for h in range(HPC) for jq in range(SQ // 512)]
            kt_queue = [(1, j) for j in range(4)] + [(2, j) for j in range(4)]

            for bi, (jq, h) in enumerate(blocks):
                qt_unit(h, jq)
                cx = CP.tile([65, 512], f32, tag="ctx", name="cx")
                # g-pairs: batch same-PE-tiling-mode matmuls to minimize
                # tiling-mode switch drains (scores are 64-row mode, ctx is
                # 128x128)
                for gg in range(8):
                    emit_out_stage()
                    # interleave remaining projection work into the
                    # ScalarE-bound steady state
                    if bi == 0:
                        for t in range(4 * gg, 4 * gg + 4):
                            v_unit(t)
                        if gg in (0, 2, 4):
                            kt_unit(0, gg // 2 + 1)
                    elif gg in (2, 5) and kt_queue:
                        kt_unit(*kt_queue.pop(0))
                    if gg == 4 and bi + 1 < len(blocks):
                        njq, nh = blocks[bi + 1]
                        qt_unit(nh, njq)
                    if gg == 6 and bi + 2 < len(blocks):
                        njq, nh = blocks[bi + 2]
                        qt_unit(nh, njq)

                    scs = []
                    for g in (2 * gg, 2 * gg + 1):
                        sc = BP.tile([128, 1024], f32, tag="big", name="sc")
                        scs.append(sc)
                        nc.tensor.matmul(
                            sc[:, 0:512],
                            kt[h][0:64, g * 128 : (g + 1) * 128],
                            qt[h][0:64, jq * 512 : (jq + 1) * 512],
                            start=True,
                            stop=True,
                        )
                        nc.tensor.matmul(
                            sc[:, 512:1024],
                            kt[h][64:128, g * 128 : (g + 1) * 128],
                            qt[h][64:128, jq * 512 : (jq + 1) * 512],
                            start=True,
                            stop=True,
                        )
                    pts = []
                    for sc in scs:
                        pt = WK.tile([128, 1024], bf16, tag="pts", name="pt")
                        pts.append(pt)
                        nc.scalar.activation(pt[:], sc[:], EXP, scale=0.125)
                    for gi, g in enumerate((2 * gg, 2 * gg + 1)):
                        pt = pts[gi]
                        nc.tensor.matmul(
                            cx[:],
                            vv[:, (2 * g) * VC + h * 65 : (2 * g) * VC + h * 65 + 65],
                            pt[:, 0:512],
                            start=(g == 0),
                            stop=False,
                        )
                        nc.tensor.matmul(
                            cx[:],
                            vv[
                                :,
                                (2 * g + 1) * VC
                                + h * 65 : (2 * g + 1) * VC
                                + h * 65
                                + 65,
                            ],
                            pt[:, 512:1024],
                            start=False,
                            stop=(g == 15),
                        )
                out_stage_q.append((jq, h, cx, {"step": 0}))
            flush_out_stages()

    nc.compile()
    return nc


def _get_nc():
    if "nc" not in _CACHE:
        _CACHE["nc"] = _build()
    return _CACHE["nc"]


def _in_maps(hs, mask, Wq, bq, Wk, bk, Wv, bv):
    ident = np.eye(128, dtype=np.float32)
    maskt = np.ascontiguousarray(mask.reshape(NT, 128).T)  # [128, 32]
    hsT = np.ascontiguousarray(hs.astype(bf16np).T)  # [768, 4096] bf16
    hsqT = [
        np.ascontiguousarray(hs[sh * SQ : (sh + 1) * SQ, :].astype(bf16np).T)
        for sh in range(QS)
    ]

    def chunk_major(W, cols):  # [768, ncols] f32 -> [128, 6*ncols] bf16
        out = np.empty((128, NHC * cols), bf16np)
        for c in range(NHC):
            out[:, c * cols : (c + 1) * cols] = W[c * 128 : (c + 1) * 128, :].astype(
                bf16np
            )
        return out

    maps = []
    for core in range(N_CORES):
        hg, sh = core // QS, core % QS
        csl = slice(hg * CC, (hg + 1) * CC)
        wv_aug = np.zeros((HID, VC), np.float32)
        bv_aug = np.zeros((1, VC), np.float32)
        for h in range(HPC):
            wv_aug[:, h * 65 : h * 65 + 64] = Wv[
                :, hg * CC + h * 64 : hg * CC + (h + 1) * 64
            ]
            bv_aug[0, h * 65 : h * 65 + 64] = bv[
                hg * CC + h * 64 : hg * CC + (h + 1) * 64
            ]
            bv_aug[0, h * 65 + 64] = 1.0
        bqt = np.empty((128, HPC), np.float32)
        bkt = np.empty((128, HPC), np.float32)
        for h in range(HPC):
            bqt[0:64, h] = bq[hg * CC + h * 64 : hg * CC + (h + 1) * 64]
            bqt[64:128, h] = bqt[0:64, h]
            bkt[0:64, h] = bk[hg * CC + h * 64 : hg * CC + (h + 1) * 64]
            bkt[64:128, h] = bkt[0:64, h]
        maps.append(
            {
                "hsT": hsT,
                "hsqT": hsqT[sh],
                "wqb": chunk_major(Wq[:, csl], CC),
                "wkb": chunk_major(Wk[:, csl], CC),
                "wvb": chunk_major(wv_aug, VC),
                "bqt": bqt,
                "bkt": bkt,
                "bvb": bv_aug.astype(bf16np),
                "maskt": maskt,
                "ident": ident,
            }
        )
    return maps


def kernel(hidden_states, attention_mask, Wq, bq, Wk, bk, Wv, bv, **run_kwargs):
    hs = np.ascontiguousarray(np.asarray(hidden_states, np.float32).reshape(S, HID))
    mask = np.ascontiguousarray(np.asarray(attention_mask, np.float32).reshape(S))
    Wq = np.asarray(Wq, np.float32)
    Wk = np.asarray(Wk, np.float32)
    Wv = np.asarray(Wv, np.float32)
    bq = np.asarray(bq, np.float32)
    bk = np.asarray(bk, np.float32)
    bv = np.asarray(bv, np.float32)

    nc = _get_nc()
    maps = _in_maps(hs, mask, Wq, bq, Wk, bk, Wv, bv)
    res = bass_utils.run_bass_kernel_spmd(
        nc, maps, core_ids=list(range(N_CORES)), **run_kwargs
    )
    out = np.zeros((S, NH * HD), np.float32)
    for core in range(N_CORES):
        hg, sh = core // QS, core % QS
        out[sh * SQ : (sh + 1) * SQ, hg * CC : (hg + 1) * CC] = res.results[core][
            "out"
        ]
    if "trace" in run_kwargs:
        _CACHE["last_result"] = res
    return out.reshape(B, S, NH * HD)
